# revision 6
# baseline (speedup 1.0000x reference)
# Trainium2 Bass kernel for nn_Member_Aggregator (GNN attention aggregation).
#
# Math (per edge e with node n = segment(e), 32 edges/node):
#   e_u   = u2e[neigh_idx]                          [E, 64]
#   g_rep = g2e[nodes][seg]                         [E, 64]
#   h1    = relu(e_u @ W1a.T + g_rep @ W1b.T + b1)  [E, 64]   (att1_w = [W1a | W1b])
#   h2    = relu(h1 @ W2.T + b2)                    [E, 64]
#   lg    = h2 @ w3.T (+ b3, dropped: softmax-invariant)
#   att   = segment_softmax(lg); out[n] = sum att * e_u        [N, 64]
#
# Sharding: 5000 contiguous nodes per core (x8), tables+weights replicated.
#
# Gather strategy: the custom InstDMAGatherAnt (gpsimd.dma_gather) is ~2
# orders of magnitude cheaper per row than indirect_dma_start, but its
# indices are int16 (<32768) while u2e has 100k rows.  So edge gathers run
# in two passes through a DRAM bounce:
#   Stage A (per group of NBG blocks): edges sorted by u2e quarter-table
#     chunk; 4x dma_gather (one per 25k-row chunk, chunk-relative indices)
#     into a compact SBUF buffer in sorted order, then one contiguous DMA
#     to a DRAM scratch tile.
#   Stage B (per block): one dma_gather from the group's scratch rows with
#     group-local permutation indices (< 4*P_A <= 32k) that lands each
#     edge's f32 row exactly in the block/tile slot layout below.
# The q (group-embedding) gather uses g2e viewed as [25000, 128] pairs so
# idx = gid//2 fits int16; the correct half is selected on DVE with
# per-partition 0/1 masks, rebuilding the paired [64, 2*64] "gt" layout.
#
# Per-core layout ("stacked" feature-major): nodes padded to 5120 = 40 blocks
# x 128 nodes. Block = 4 tiles x 1024 edges. A tile pairs nodes {16t..16t+15}
# (top, SBUF partitions 0..63) with {64+16t..} (bottom, partitions 64..127),
# so every [128, 512] activation column holds one top edge + one bottom edge
# and all matmuls use block-diagonal weights at full 128-partition width.
# Edge slot x in [0,1024): x = c*128 + p (gather chunk c, partition p);
# top x = 32*j + k (node-slot j, neighbor k), bottom x-512 likewise.
#
# Per-edge q = g_rep @ W1b.T + b1 is folded into mm1 as extra contraction rows
# (lhsT = transposed per-node q, rhs = constant node-indicator), so no
# per-edge vector add is needed.

import os
import sys

import numpy as np

for _p in ("/opt/trn_rl_repo",):
    if _p not in sys.path:
        sys.path.insert(0, _p)

N_NODES = 40000
DEG = 32
D = 64
NUM_USERS = 100000
NUM_GROUPS = 50000
N_CORES = 8
NPC = N_NODES // N_CORES  # 5000 nodes per core
TPB = 4                   # tiles per block
EPT = 1024                # edges per tile
NBG = 5                   # blocks per stage-A group
UCH = NUM_USERS // 4      # u2e chunk rows (25000 < 32768)

_cache = {}


def _groups(nblk):
    """Split nblk blocks into groups of <= NBG blocks."""
    out = []
    b = 0
    while b < nblk:
        out.append(list(range(b, min(b + NBG, nblk))))
        b += NBG
    return out


def _build_program(nblk, pa):
    """SPMD per-core Bass program for `nblk` 128-node blocks; `pa` = padded
    per-(group,chunk) stage-A gather size (multiple of 128)."""
    import concourse.bass as bass
    import concourse.tile as tile
    from concourse import bacc, mybir
    from contextlib import ExitStack

    f32 = mybir.dt.float32
    bf16 = mybir.dt.bfloat16
    i16 = mybir.dt.int16
    AF = mybir.ActivationFunctionType
    ALU = mybir.AluOpType
    AX = mybir.AxisListType

    groups = _groups(nblk)
    ng = len(groups)
    sc = pa // 128            # compact cols per chunk
    pa16 = pa // 16           # idx cols per chunk

    nc = bacc.Bacc("TRN2", target_bir_lowering=False, debug=False,
                   num_devices=N_CORES)

    u2ef = nc.dram_tensor("u2ef", [NUM_USERS, D], f32, kind="ExternalInput").ap()
    g2ep = nc.dram_tensor("g2ep", [NUM_GROUPS // 2, 2 * D], f32,
                          kind="ExternalInput").ap()
    ea_d = nc.dram_tensor("ea", [ng * 128, 4 * pa16], i16,
                          kind="ExternalInput").ap()
    eb_d = nc.dram_tensor("eb", [nblk * 128, EPT * TPB // 16], i16,
                          kind="ExternalInput").ap()
    gq_d = nc.dram_tensor("gq", [ng * 128, 16 * NBG], i16,
                          kind="ExternalInput").ap()
    qm_d = nc.dram_tensor("qm", [64, nblk * 4], f32, kind="ExternalInput").ap()
    w1a_d = nc.dram_tensor("w1a", [128, 128], bf16, kind="ExternalInput").ap()
    w1b_d = nc.dram_tensor("w1b", [128, 128], f32, kind="ExternalInput").ap()
    w2_d = nc.dram_tensor("w2", [128, 128], bf16, kind="ExternalInput").ap()
    w3_d = nc.dram_tensor("w3q", [128, TPB * 8], bf16, kind="ExternalInput").ap()
    ones_d = nc.dram_tensor("onesbd", [2, 128], bf16, kind="ExternalInput").ap()
    ind_d = nc.dram_tensor("ind64", [64, TPB * 512], bf16,
                           kind="ExternalInput").ap()
    b1_d = nc.dram_tensor("b1st", [128, 1], f32, kind="ExternalInput").ap()
    b2_d = nc.dram_tensor("b2st", [128, 1], f32, kind="ExternalInput").ap()
    id_d = nc.dram_tensor("ident", [128, 128], f32, kind="ExternalInput").ap()
    outd = nc.dram_tensor("out", [nblk * 128, D], f32, kind="ExternalOutput").ap()

    with tile.TileContext(nc) as tc, ExitStack() as ctx:
        cp = ctx.enter_context(tc.tile_pool(name="consts", bufs=1))

        def load_const(dram_ap, shape, tag, dt=f32):
            t = cp.tile(shape, dt, tag=tag)
            nc.sync.dma_start(t[:], dram_ap)
            return t

        w1a_t = load_const(w1a_d, [128, 128], "w1a", bf16)
        w1b_t = load_const(w1b_d, [128, 128], "w1b")
        w2_t = load_const(w2_d, [128, 128], "w2", bf16)
        w3_t = load_const(w3_d, [128, TPB * 8], "w3", bf16)
        ones_t = load_const(ones_d, [2, 128], "ones", bf16)
        ind_t = load_const(ind_d, [64, TPB * 512], "ind", bf16)
        b1_t = load_const(b1_d, [128, 1], "b1")
        b2_t = load_const(b2_d, [128, 1], "b2")
        id_t = load_const(id_d, [128, 128], "ident")
        qm_t = load_const(qm_d, [64, nblk * 4], "qm")

        # stage A pools
        eap = ctx.enter_context(tc.tile_pool(name="ea", bufs=2))
        cmp_p = ctx.enter_context(tc.tile_pool(name="cmp", bufs=2))
        scr_p = ctx.enter_context(tc.tile_pool(name="scr", bufs=3,
                                               space="DRAM"))
        gqp = ctx.enter_context(tc.tile_pool(name="gqi", bufs=2))
        qprp = ctx.enter_context(tc.tile_pool(name="qpr", bufs=2))
        # main loop pools
        ebp = ctx.enter_context(tc.tile_pool(name="eb", bufs=3))
        gep = ctx.enter_context(tc.tile_pool(name="ge", bufs=2))
        gq = ctx.enter_context(tc.tile_pool(name="gsel", bufs=2))
        qps = ctx.enter_context(tc.tile_pool(name="qpsum", bufs=2, space="PSUM"))
        tpps = ctx.enter_context(tc.tile_pool(name="tp", bufs=2, space="PSUM"))
        eut = ctx.enter_context(tc.tile_pool(name="eut", bufs=6))
        mmps = ctx.enter_context(tc.tile_pool(name="mm", bufs=2, space="PSUM"))
        hsb = ctx.enter_context(tc.tile_pool(name="h", bufs=3))
        lgps = ctx.enter_context(tc.tile_pool(name="lg", bufs=1, space="PSUM"))
        abps = ctx.enter_context(tc.tile_pool(name="attb", bufs=1, space="PSUM"))
        lrow_p = ctx.enter_context(tc.tile_pool(name="lrow", bufs=2))
        arow_p = ctx.enter_context(tc.tile_pool(name="arow", bufs=2))
        nm = ctx.enter_context(tc.tile_pool(name="nm", bufs=2))
        wsb_p = ctx.enter_context(tc.tile_pool(name="w", bufs=2))
        wacc_p = ctx.enter_context(tc.tile_pool(name="wacc", bufs=2))
        osb_p = ctx.enter_context(tc.tile_pool(name="osb", bufs=2))

        # ---- stage A: chunk-sorted edge gathers -> DRAM scratch; q pairs ----
        scr_tiles = []
        qpr_tiles = []
        for g, blocks in enumerate(groups):
            gb = len(blocks)
            ea_t = eap.tile([128, 4 * pa16], i16)
            nc.sync.dma_start(ea_t[:], ea_d[g * 128:(g + 1) * 128, :])
            cmp_t = cmp_p.tile([128, 4 * sc * D], f32)
            for c in range(4):
                nc.gpsimd.dma_gather(
                    out_ap=cmp_t[:, c * sc * D:(c + 1) * sc * D]
                        .rearrange("p (s e) -> p s e", e=D),
                    in_ap=u2ef[c * UCH:(c + 1) * UCH, :],
                    idxs_ap=ea_t[:, c * pa16:(c + 1) * pa16],
                    num_idxs=pa, num_idxs_reg=pa, elem_size=D)
            scr_t = scr_p.tile([128, 4 * sc * D], f32)
            nc.sync.dma_start(scr_t[:], cmp_t[:])
            scr_tiles.append(scr_t)

            gq_t = gqp.tile([128, 16 * gb], i16)
            nc.sync.dma_start(gq_t[:], gq_d[g * 128:(g + 1) * 128, 0:16 * gb])
            qpr_t = qprp.tile([128, 2 * gb, 2 * D], f32)
            nc.gpsimd.dma_gather(
                out_ap=qpr_t[:], in_ap=g2ep, idxs_ap=gq_t[:],
                num_idxs=128 * 2 * gb, num_idxs_reg=128 * 2 * gb,
                elem_size=2 * D)
            qpr_tiles.append(qpr_t)

        # ---- main loop over blocks ----
        for b in range(nblk):
            g = b // NBG
            bg = b - g * NBG
            scr_t = scr_tiles[g]
            qpr_t = qpr_tiles[g]

            # q phase: rebuild paired gt [64, 2*64] = [g(n) | g(n+64)] via
            # parity masks, then transpose as in the dense-gather version.
            gt = gq.tile([64, 128], f32, tag="gt")
            tq1 = gq.tile([64, D], f32, tag="tq1")
            tq2 = gq.tile([64, D], f32, tag="tq2")
            for h in range(2):
                nc.vector.tensor_scalar_mul(
                    tq1[:], qpr_t[0:64, 2 * bg + h, 0:D],
                    qm_t[:, 4 * b + 2 * h:4 * b + 2 * h + 1])
                nc.vector.tensor_scalar_mul(
                    tq2[:], qpr_t[0:64, 2 * bg + h, D:2 * D],
                    qm_t[:, 4 * b + 2 * h + 1:4 * b + 2 * h + 2])
                nc.vector.tensor_tensor(out=gt[:, D * h:D * (h + 1)],
                                        in0=tq1[:], in1=tq2[:], op=ALU.add)
            g2T = qps.tile([128, 128], f32, tag="qp")
            nc.tensor.transpose(out=g2T[:, 0:64], in_=gt[:],
                                identity=id_t[0:64, 0:64])
            g2T_sb = gq.tile([128, D], f32, tag="g2Tsb")
            nc.scalar.copy(g2T_sb[:], g2T[:, 0:64])
            qp = qps.tile([128, 128], f32, tag="qp")
            nc.tensor.matmul(qp[:, 0:64], lhsT=w1b_t[:], rhs=g2T_sb[:],
                             start=True, stop=True)
            q2T_sb = gq.tile([128, D], f32, tag="q2T")
            nc.vector.tensor_scalar_add(q2T_sb[:], qp[:, 0:64], b1_t[:, :1])
            qT2p = qps.tile([128, 128], f32, tag="qp")
            nc.tensor.transpose(out=qT2p[0:64, :], in_=q2T_sb[:], identity=id_t[:])
            qT2_sb = gq.tile([64, 128], bf16, tag="qT2")
            nc.scalar.copy(qT2_sb[:], qT2p[0:64, :])

            # ---- edge phase: one permuted gather from scratch per block ----
            eb_t = ebp.tile([128, EPT * TPB // 16], i16)
            nc.sync.dma_start(eb_t[:], eb_d[b * 128:(b + 1) * 128, :])
            ge = gep.tile([128, TPB * 8 * D], f32)
            nc.gpsimd.dma_gather(
                out_ap=ge[:].rearrange("p (s e) -> p s e", e=D),
                in_ap=scr_t[:].rearrange("p (r e) -> (p r) e", e=D),
                idxs_ap=eb_t[:],
                num_idxs=TPB * EPT, num_idxs_reg=TPB * EPT, elem_size=D)

            lg8 = lgps.tile([8, 512], f32)
            euts = []
            for t in range(TPB):
                # stacked transpose: [128, 128] f32 blocks pair chunks
                # (2u, 2u+1) into top/bottom feature halves.
                tp = tpps.tile([128, 512], f32)
                for u in range(4):
                    nc.tensor.transpose(
                        out=tp[:, 128 * u:128 * (u + 1)],
                        in_=ge[:, (t * 8 + 2 * u) * D:(t * 8 + 2 * u + 2) * D],
                        identity=id_t[:])
                eut_sb = eut.tile([128, 512], bf16)
                nc.scalar.copy(eut_sb[:], tp[:])
                euts.append(eut_sb)

                h1p = mmps.tile([128, 512], f32, tag="mm")
                nc.tensor.matmul(h1p[:], lhsT=(w1a_t[:]),
                                 rhs=(eut_sb[:]), start=True, stop=False)
                nc.tensor.matmul(h1p[:], lhsT=(qT2_sb[:]),
                                 rhs=(ind_t[:, t * 512:(t + 1) * 512]),
                                 start=False, stop=True)
                h1sb = hsb.tile([128, 512], bf16, tag="h")
                nc.scalar.activation(h1sb[:], h1p[:], AF.Relu)
                h2p = mmps.tile([128, 512], f32, tag="mm")
                nc.tensor.matmul(h2p[:], lhsT=(w2_t[:]),
                                 rhs=(h1sb[:]), start=True, stop=True)
                h2sb = hsb.tile([128, 512], bf16, tag="h")
                nc.scalar.activation(h2sb[:], h2p[:], AF.Relu, bias=b2_t[:, :1])
                nc.tensor.matmul(lg8[:], lhsT=(w3_t[:, 8 * t:8 * (t + 1)]),
                                 rhs=(h2sb[:]), start=(t == 0),
                                 stop=(t == TPB - 1))

            # ---- softmax over each node's 32 edges (node-major [128, 32]) ----
            lrow = lrow_p.tile([8, 512], f32)
            nc.scalar.copy(lrow[:], lg8[:])
            lnm = nm.tile([128, 32], f32, tag="lnm")
            for t in range(TPB):
                for h in range(2):
                    nc.gpsimd.dma_start(
                        lnm[64 * h + 16 * t:64 * h + 16 * t + 16, :],
                        lrow[2 * t + h:2 * t + h + 1, :]
                            .rearrange("p (j k) -> p j k", j=16))
            ngmax = nm.tile([128, 1], f32, tag="ngmax")
            nc.vector.tensor_reduce(out=ngmax[:], in_=lnm[:], axis=AX.X,
                                    op=ALU.max, negate=True)
            expn = nm.tile([128, 32], f32, tag="expn")
            sume = nm.tile([128, 1], f32, tag="sume")
            nc.scalar.activation(expn[:], lnm[:], AF.Exp, bias=ngmax[:, :1],
                                 accum_out=sume[:, :1])
            rinv = nm.tile([128, 1], f32, tag="rinv")
            nc.vector.reciprocal(rinv[:], sume[:])
            attn = nm.tile([128, 32], bf16, tag="attn")
            nc.vector.tensor_scalar_mul(attn[:], expn[:], rinv[:, :1])
            arow = arow_p.tile([2, TPB * 512], bf16)
            for t in range(TPB):
                for h in range(2):
                    nc.gpsimd.dma_start(
                        arow[h:h + 1, 512 * t:512 * (t + 1)]
                            .rearrange("p (j k) -> p j k", j=16),
                        attn[64 * h + 16 * t:64 * h + 16 * t + 16, :])

            # ---- weighted aggregation ----
            wacc = wacc_p.tile([128, D], f32)
            for t in range(TPB):
                ab = abps.tile([128, 512], f32)
                nc.tensor.matmul(ab[:], lhsT=(ones_t[:]),
                                 rhs=(arow[:, t * 512:(t + 1) * 512]),
                                 start=True, stop=True)
                wt = wsb_p.tile([128, 512], f32)
                nc.vector.tensor_tensor(out=wt[:], in0=euts[t][:], in1=ab[:],
                                        op=ALU.mult)
                nc.vector.tensor_reduce(
                    out=wacc[:, 16 * t:16 * (t + 1)],
                    in_=wt[:].rearrange("p (j k) -> p j k", j=16),
                    axis=AX.X, op=ALU.add)
            outp = qps.tile([128, 128], f32, tag="qp")
            nc.tensor.transpose(out=outp[0:64, :], in_=wacc[:], identity=id_t[:])
            osb = osb_p.tile([64, 128], f32)
            nc.scalar.copy(osb[:], outp[0:64, :])
            nc.gpsimd.dma_start(
                outd[b * 128:(b + 1) * 128, :]
                    .rearrange("(pair n) d -> n pair d", pair=2),
                osb[:].rearrange("n (pair d) -> n pair d", pair=2))

    nc.compile()
    return nc


def _wrap16(arr):
    """[n] -> [128, n/16] int16 idx layout (16-partition wrap, replicated)."""
    n = arr.shape[0]
    w = arr.reshape(n // 16, 16).T.astype(np.int16)
    return np.ascontiguousarray(np.tile(w, (8, 1)))


def _prep_host(nodes, neigh_idx, att1_w, att1_b, att2_w, att2_b, att3_w,
               nblk_per_core):
    """Shard + reorder indices, build constant tensors. Returns per-core maps
    (without the shared tables) and the stage-A pad size pa."""
    npad = nblk_per_core * 128
    npc = min(NPC, npad)
    nodes = np.asarray(nodes).astype(np.int32)
    neigh = np.asarray(neigh_idx).astype(np.int32).reshape(-1, DEG)
    groups = _groups(nblk_per_core)
    ng = len(groups)

    consts = {}
    att1_w = np.asarray(att1_w, np.float32)
    w1aT = att1_w[:, :D].T.copy()
    w1bT = att1_w[:, D:].T.copy()
    w2T = np.asarray(att2_w, np.float32).T.copy()

    def blockdiag(m):
        z = np.zeros((128, 128), np.float32)
        z[:64, :64] = m
        z[64:, 64:] = m
        return z

    import ml_dtypes
    bf = ml_dtypes.bfloat16
    consts["w1a"] = blockdiag(w1aT).astype(bf)
    consts["w1b"] = blockdiag(w1bT)
    consts["w2"] = blockdiag(w2T).astype(bf)
    # w3q[:, t*8 + 2t + h] = w3 half-h; tile t's mm3 writes lg8 rows 2t, 2t+1
    w3q = np.zeros((128, TPB, 8), np.float32)
    w3row = np.asarray(att3_w, np.float32)[0]
    for t in range(TPB):
        w3q[:64, t, 2 * t] = w3row
        w3q[64:, t, 2 * t + 1] = w3row
    consts["w3q"] = w3q.reshape(128, TPB * 8).astype(bf)
    ones_bd = np.zeros((2, 128), np.float32)
    ones_bd[0, :64] = 1.0
    ones_bd[1, 64:] = 1.0
    consts["onesbd"] = ones_bd.astype(bf)
    # ind64[j, t*512 + e] = 1 iff j == 16t + e//32 (mm1b scatters per-node q)
    ind64 = np.zeros((64, TPB * 512), np.float32)
    for t in range(TPB):
        ind64[16 * t:16 * (t + 1), 512 * t:512 * (t + 1)] = np.repeat(
            np.eye(16, dtype=np.float32), 32, axis=1)
    consts["ind64"] = ind64.astype(bf)
    consts["b1st"] = np.tile(np.asarray(att1_b, np.float32), 2)[:, None].copy()
    consts["b2st"] = np.tile(np.asarray(att2_b, np.float32), 2)[:, None].copy()
    consts["ident"] = np.eye(128, dtype=np.float32)

    ncores = len(nodes) // npc if len(nodes) >= npc else 1

    # ---- per-core slot->uid map in gather-position order ----
    # slot order within a block: position = (8t + 2u4 + h)*128 + p, matching
    # the in-tile layout x = c*128 + p with c = 2*u4 + h.
    def slot_uids(nix):
        a = nix.reshape(nblk_per_core, 2, TPB, 16, DEG).transpose(0, 2, 1, 3, 4)
        a = a.reshape(nblk_per_core, TPB, 2, 4, 128)  # [b, t, h, u4, p]
        a = a.transpose(0, 1, 3, 2, 4)                # [b, t, u4, h, p]
        return np.ascontiguousarray(a).reshape(-1)

    # first pass: compute pa (max per-(core,group,chunk) count, padded)
    pa = 0
    per_core_uids = []
    for c in range(ncores):
        n0 = c * npc
        nix = np.zeros((npad, DEG), np.int32)
        nix[:npc] = neigh[n0:n0 + npc]
        uids = slot_uids(nix)
        per_core_uids.append(uids)
        for g, blocks in enumerate(groups):
            ge = uids[blocks[0] * TPB * EPT:
                      (blocks[-1] + 1) * TPB * EPT]
            cnt = np.bincount(ge // UCH, minlength=4).max()
            pa = max(pa, int(cnt))
    pa = ((pa + 127) // 128) * 128
    sc = pa // 128

    per_core = []
    for c in range(ncores):
        n0 = c * npc
        gid = np.zeros(npad, np.int32)
        gid[:npc] = nodes[n0:n0 + npc]
        uids = per_core_uids[c]

        ea = np.zeros((ng * 128, 4 * (pa // 16)), np.int16)
        eb = np.zeros((nblk_per_core * 128, TPB * EPT // 16), np.int16)
        for g, blocks in enumerate(groups):
            e0 = blocks[0] * TPB * EPT
            e1 = (blocks[-1] + 1) * TPB * EPT
            u = uids[e0:e1]
            ch = u // UCH
            rel = (u % UCH).astype(np.int16)
            order = np.argsort(ch, kind="stable")
            ranks = np.empty(u.shape[0], np.int64)
            ranks[order] = np.arange(u.shape[0])
            counts = np.bincount(ch, minlength=4)
            cum = np.concatenate([[0], np.cumsum(counts)])
            r_in = ranks - cum[ch]
            gpos = ch * pa + r_in            # padded sorted position
            ea_list = np.zeros(4 * pa, np.int16)
            ea_list[gpos] = rel
            # scratch row of padded sorted position s:
            j = gpos % pa
            cpos = gpos // pa
            srow = (j % 128) * (4 * sc) + cpos * sc + j // 128
            ea[g * 128:(g + 1) * 128] = _wrap16(ea_list)
            srow = srow.reshape(len(blocks), TPB * EPT)
            for bi, b in enumerate(blocks):
                eb[b * 128:(b + 1) * 128] = _wrap16(srow[bi].astype(np.int16))

        # q pair idx: per group, position 128*(2*bg+h) + p -> pair id of
        # node 64*h + p%64 of block b (p >= 64 duplicates p-64).
        gq = np.zeros((ng * 128, 16 * NBG), np.int16)
        qmask = np.zeros((64, nblk_per_core, 4), np.float32)
        gid_b = gid.reshape(nblk_per_core, 2, 64)  # [b, h, j]
        for g, blocks in enumerate(groups):
            gb = len(blocks)
            ql = np.zeros((2 * gb) * 128, np.int16)
            for bi, b in enumerate(blocks):
                for h in range(2):
                    col = gid_b[b, h] // 2
                    ql[(2 * bi + h) * 128:(2 * bi + h) * 128 + 64] = col
                    ql[(2 * bi + h) * 128 + 64:(2 * bi + h + 1) * 128] = col
            gq[g * 128:(g + 1) * 128, 0:16 * gb] = _wrap16(ql)
        for b in range(nblk_per_core):
            for h in range(2):
                par = (gid_b[b, h] % 2).astype(np.float32)
                qmask[:, b, 2 * h] = 1.0 - par
                qmask[:, b, 2 * h + 1] = par

        m = dict(consts)
        m["ea"] = np.ascontiguousarray(ea)
        m["eb"] = np.ascontiguousarray(eb)
        m["gq"] = np.ascontiguousarray(gq)
        m["qm"] = np.ascontiguousarray(qmask.reshape(64, nblk_per_core * 4))
        per_core.append(m)
    return per_core, pa


def kernel(nodes, neigh_idx, segment_ids, u2e_weight, g2e_weight,
           att1_w, att1_b, att2_w, att2_b, att3_w, att3_b):
    from concourse import bass_utils

    nblk = NPC // 128 + (1 if NPC % 128 else 0)  # 40
    per_core, pa = _prep_host(nodes, neigh_idx, att1_w, att1_b, att2_w,
                              att2_b, att3_w, nblk)
    key = ("prog", nblk, pa)
    if key not in _cache:
        _cache[key] = _build_program(nblk, pa)
    nc = _cache[key]

    u2ef = np.ascontiguousarray(np.asarray(u2e_weight, np.float32))
    g2ep = np.ascontiguousarray(
        np.asarray(g2e_weight, np.float32).reshape(NUM_GROUPS // 2, 2 * D))
    in_maps = []
    for m in per_core:
        m = dict(m)
        m["u2ef"] = u2ef
        m["g2ep"] = g2ep
        in_maps.append(m)

    res = bass_utils.run_bass_kernel_spmd(nc, in_maps,
                                          core_ids=list(range(N_CORES)))
    outs = [np.asarray(r["out"])[:NPC] for r in res.results]
    return np.concatenate(outs, axis=0)


# revision 16
# speedup vs baseline: 1.4036x; 1.4036x over previous
# Trainium2 Bass kernel for nn_Member_Aggregator (GNN attention aggregation).
#
# Math (per edge e with node n = segment(e), 32 edges/node):
#   e_u   = u2e[neigh_idx]                          [E, 64]
#   g_rep = g2e[nodes][seg]                         [E, 64]
#   h1    = relu(e_u @ W1a.T + g_rep @ W1b.T + b1)  [E, 64]   (att1_w = [W1a | W1b])
#   h2    = relu(h1 @ W2.T + b2)                    [E, 64]
#   lg    = h2 @ w3.T (+ b3, dropped: softmax-invariant)
#   att   = segment_softmax(lg); out[n] = sum att * e_u        [N, 64]
#
# Sharding: 5000 contiguous nodes per core (x8), tables+weights replicated.
#
# Gather strategy: the custom InstDMAGatherAnt (gpsimd.dma_gather) is ~2
# orders of magnitude cheaper per row than indirect_dma_start, but its
# indices are int16 (<32768) while u2e has 100k rows.  So edge gathers run
# in two passes through a DRAM bounce:
#   Stage A (per group of NBG blocks): edges sorted by u2e quarter-table
#     chunk; 4x dma_gather (one per 25k-row chunk, chunk-relative indices)
#     into a compact SBUF buffer in sorted order, then one contiguous DMA
#     to a DRAM scratch tile.
#   Stage B (per block): one dma_gather from the group's scratch rows with
#     group-local permutation indices (< 4*P_A <= 32k) that lands each
#     edge's f32 row exactly in the block/tile slot layout below.
# The q (group-embedding) gather uses g2e viewed as [25000, 128] pairs so
# idx = gid//2 fits int16; the correct half is selected on DVE with
# per-partition 0/1 masks, rebuilding the paired [64, 2*64] "gt" layout.
#
# Per-core layout ("stacked" feature-major): nodes padded to 5120 = 40 blocks
# x 128 nodes. Block = 4 tiles x 1024 edges. A tile pairs nodes {16t..16t+15}
# (top, SBUF partitions 0..63) with {64+16t..} (bottom, partitions 64..127),
# so every [128, 512] activation column holds one top edge + one bottom edge
# and all matmuls use block-diagonal weights at full 128-partition width.
# Edge slot x in [0,1024): x = c*128 + p (gather chunk c, partition p);
# top x = 32*j + k (node-slot j, neighbor k), bottom x-512 likewise.
#
# Per-edge q = g_rep @ W1b.T + b1 is folded into mm1 as extra contraction rows
# (lhsT = transposed per-node q, rhs = constant node-indicator), so no
# per-edge vector add is needed.

import os
import sys

import numpy as np

for _p in ("/opt/trn_rl_repo",):
    if _p not in sys.path:
        sys.path.insert(0, _p)

N_NODES = 40000
DEG = 32
D = 64
NUM_USERS = 100000
NUM_GROUPS = 50000
N_CORES = 8
NPC = N_NODES // N_CORES  # 5000 nodes per core
TPB = 4                   # tiles per block
EPT = 1024                # edges per tile
NBG = 5                   # blocks per stage-A group
UCH = NUM_USERS // 4      # u2e chunk rows (25000 < 32768)

_cache = {}


def _groups(nblk):
    """Split nblk blocks into groups of <= NBG blocks."""
    out = []
    b = 0
    while b < nblk:
        out.append(list(range(b, min(b + NBG, nblk))))
        b += NBG
    return out


def _build_program(nblk, pa):
    """SPMD per-core Bass program for `nblk` 128-node blocks; `pa` = padded
    per-(group,chunk) stage-A gather size (multiple of 128)."""
    import concourse.bass as bass
    import concourse.tile as tile
    from concourse import bacc, mybir
    from contextlib import ExitStack

    f32 = mybir.dt.float32
    bf16 = mybir.dt.bfloat16
    i16 = mybir.dt.int16
    AF = mybir.ActivationFunctionType
    ALU = mybir.AluOpType
    AX = mybir.AxisListType

    groups = _groups(nblk)
    ng = len(groups)
    sc = pa // 128            # compact cols per chunk
    pa16 = pa // 16           # idx cols per chunk

    nc = bacc.Bacc("TRN2", target_bir_lowering=False, debug=False,
                   num_devices=N_CORES)

    u2ef = nc.dram_tensor("u2ef", [NUM_USERS, D], f32, kind="ExternalInput").ap()
    g2ep = nc.dram_tensor("g2ep", [NUM_GROUPS // 2, 2 * D], f32,
                          kind="ExternalInput").ap()
    ea_d = nc.dram_tensor("ea", [ng * 128, 4 * pa16], i16,
                          kind="ExternalInput").ap()
    eb_d = nc.dram_tensor("eb", [nblk * 128, EPT * TPB // 16], i16,
                          kind="ExternalInput").ap()
    gq_d = nc.dram_tensor("gq", [ng * 128, 16 * NBG], i16,
                          kind="ExternalInput").ap()
    qm_d = nc.dram_tensor("qm", [64, nblk * 4], f32, kind="ExternalInput").ap()
    w1a_d = nc.dram_tensor("w1a", [128, 128], bf16, kind="ExternalInput").ap()
    w1b_d = nc.dram_tensor("w1b", [128, 128], f32, kind="ExternalInput").ap()
    w2_d = nc.dram_tensor("w2", [128, 128], bf16, kind="ExternalInput").ap()
    w3_d = nc.dram_tensor("w3q", [128, TPB * 8], bf16, kind="ExternalInput").ap()
    ones_d = nc.dram_tensor("ones8", [8, TPB * 128], bf16,
                            kind="ExternalInput").ap()
    ind_d = nc.dram_tensor("ind64", [64, TPB * 512], bf16,
                           kind="ExternalInput").ap()
    b1_d = nc.dram_tensor("b1st", [128, 1], f32, kind="ExternalInput").ap()
    b2_d = nc.dram_tensor("b2st", [128, 1], f32, kind="ExternalInput").ap()
    id_d = nc.dram_tensor("ident", [128, 128], f32, kind="ExternalInput").ap()
    outd = nc.dram_tensor("out", [nblk * 128, D], f32, kind="ExternalOutput").ap()

    with tile.TileContext(nc) as tc, ExitStack() as ctx:
        cp = ctx.enter_context(tc.tile_pool(name="consts", bufs=1))

        def load_const(dram_ap, shape, tag, dt=f32):
            t = cp.tile(shape, dt, tag=tag)
            nc.sync.dma_start(t[:], dram_ap)
            return t

        w1a_t = load_const(w1a_d, [128, 128], "w1a", bf16)
        w1b_t = load_const(w1b_d, [128, 128], "w1b")
        w2_t = load_const(w2_d, [128, 128], "w2", bf16)
        w3_t = load_const(w3_d, [128, TPB * 8], "w3", bf16)
        ones_t = load_const(ones_d, [8, TPB * 128], "ones", bf16)
        ind_t = load_const(ind_d, [64, TPB * 512], "ind", bf16)
        b1_t = load_const(b1_d, [128, 1], "b1")
        b2_t = load_const(b2_d, [128, 1], "b2")
        id_t = load_const(id_d, [128, 128], "ident")
        qm_t = load_const(qm_d, [64, nblk * 4], "qm")

        # stage A pools
        eap = ctx.enter_context(tc.tile_pool(name="ea", bufs=2))
        cmp_p = ctx.enter_context(tc.tile_pool(name="cmp", bufs=2))
        scr_p = ctx.enter_context(tc.tile_pool(name="scr", bufs=3,
                                               space="DRAM"))
        gqp = ctx.enter_context(tc.tile_pool(name="gqi", bufs=2))
        qprp = ctx.enter_context(tc.tile_pool(name="qpr", bufs=2))
        # main loop pools
        ebp = ctx.enter_context(tc.tile_pool(name="eb", bufs=3))
        gep = ctx.enter_context(tc.tile_pool(name="ge", bufs=4))
        gq = ctx.enter_context(tc.tile_pool(name="gsel", bufs=2))
        qps = ctx.enter_context(tc.tile_pool(name="qpsum", bufs=2, space="PSUM"))
        tpps = ctx.enter_context(tc.tile_pool(name="tp", bufs=2, space="PSUM"))
        eut = ctx.enter_context(tc.tile_pool(name="eut", bufs=6))
        mmps = ctx.enter_context(tc.tile_pool(name="mm", bufs=2, space="PSUM"))
        hsb = ctx.enter_context(tc.tile_pool(name="h", bufs=3))
        lgps = ctx.enter_context(tc.tile_pool(name="lg", bufs=1, space="PSUM"))
        abps = ctx.enter_context(tc.tile_pool(name="attb", bufs=1, space="PSUM"))
        nm = ctx.enter_context(tc.tile_pool(name="nm", bufs=2))
        wsb_p = ctx.enter_context(tc.tile_pool(name="w", bufs=2))
        wacc_p = ctx.enter_context(tc.tile_pool(name="wacc", bufs=2))
        osb_p = ctx.enter_context(tc.tile_pool(name="osb", bufs=2))

        # ---- stage A: chunk-sorted edge gathers -> DRAM scratch; q pairs ----
        # HW limit: a single dma_gather handles at most 1024 indices.
        scr_tiles = []
        qpr_tiles = []
        for g, blocks in enumerate(groups):
            gb = len(blocks)
            ea_t = eap.tile([128, 4 * pa16], i16)
            nc.sync.dma_start(ea_t[:], ea_d[g * 128:(g + 1) * 128, :])
            cmp_t = cmp_p.tile([128, 4 * sc * D], f32)
            for c in range(4):
                for s in range(pa // 1024):
                    col = c * sc + s * 8
                    nc.gpsimd.dma_gather(
                        out_ap=cmp_t[:, col * D:(col + 8) * D]
                            .rearrange("p (s e) -> p s e", e=D),
                        in_ap=u2ef[c * UCH:(c + 1) * UCH, :],
                        idxs_ap=ea_t[:, c * pa16 + s * 64:
                                     c * pa16 + (s + 1) * 64],
                        num_idxs=1024, num_idxs_reg=1024, elem_size=D)
            scr_t = scr_p.tile([128, 4 * sc * D], f32)
            nc.sync.dma_start(scr_t[:], cmp_t[:])
            scr_tiles.append(scr_t)

            gq_t = gqp.tile([128, 16 * gb], i16)
            nc.sync.dma_start(gq_t[:], gq_d[g * 128:(g + 1) * 128, 0:16 * gb])
            qpr_t = qprp.tile([128, 2 * gb, 2 * D], f32)
            qn = 128 * 2 * gb
            q0 = 0
            while q0 < qn:
                qc = min(1024, qn - q0)
                nc.gpsimd.dma_gather(
                    out_ap=qpr_t[:, q0 // 128:(q0 + qc) // 128, :],
                    in_ap=g2ep,
                    idxs_ap=gq_t[:, q0 // 16:(q0 + qc) // 16],
                    num_idxs=qc, num_idxs_reg=qc, elem_size=2 * D)
                q0 += qc
            qpr_tiles.append(qpr_t)

        # ---- main loop over blocks ----
        for b in range(nblk):
            g = b // NBG
            bg = b - g * NBG
            scr_t = scr_tiles[g]
            qpr_t = qpr_tiles[g]

            # q phase: rebuild paired gt [64, 2*64] = [g(n) | g(n+64)] via
            # parity masks, then transpose as in the dense-gather version.
            gt = gq.tile([64, 128], f32, tag="gt")
            tq1 = gq.tile([64, D], f32, tag="tq1")
            tq2 = gq.tile([64, D], f32, tag="tq2")
            for h in range(2):
                nc.vector.tensor_scalar_mul(
                    tq1[:], qpr_t[0:64, 2 * bg + h, 0:D],
                    qm_t[:, 4 * b + 2 * h:4 * b + 2 * h + 1])
                nc.vector.tensor_scalar_mul(
                    tq2[:], qpr_t[0:64, 2 * bg + h, D:2 * D],
                    qm_t[:, 4 * b + 2 * h + 1:4 * b + 2 * h + 2])
                nc.vector.tensor_tensor(out=gt[:, D * h:D * (h + 1)],
                                        in0=tq1[:], in1=tq2[:], op=ALU.add)
            g2T = qps.tile([128, 128], f32, tag="qp")
            nc.tensor.transpose(out=g2T[:, 0:64], in_=gt[:],
                                identity=id_t[0:64, 0:64])
            g2T_sb = gq.tile([128, D], f32, tag="g2Tsb")
            nc.scalar.copy(g2T_sb[:], g2T[:, 0:64])
            qp = qps.tile([128, 128], f32, tag="qp")
            nc.tensor.matmul(qp[:, 0:64], lhsT=w1b_t[:], rhs=g2T_sb[:],
                             start=True, stop=True)
            q2T_sb = gq.tile([128, D], f32, tag="q2T")
            nc.vector.tensor_scalar_add(q2T_sb[:], qp[:, 0:64], b1_t[:, :1])
            qT2p = qps.tile([128, 128], f32, tag="qp")
            nc.tensor.transpose(out=qT2p[0:64, :], in_=q2T_sb[:], identity=id_t[:])
            qT2_sb = gq.tile([64, 128], bf16, tag="qT2")
            nc.scalar.copy(qT2_sb[:], qT2p[0:64, :])

            # ---- edge phase: permuted gathers from scratch (one per tile) ----
            eb_t = ebp.tile([128, EPT * TPB // 16], i16)
            nc.sync.dma_start(eb_t[:], eb_d[b * 128:(b + 1) * 128, :])

            lg8 = lgps.tile([8, 512], f32)
            euts = []
            for t in range(TPB):
                ge = gep.tile([128, 8 * D], f32)
                nc.gpsimd.dma_gather(
                    out_ap=ge[:].rearrange("p (s e) -> p s e", e=D),
                    in_ap=scr_t[:].rearrange("p (r e) -> (p r) e", e=D),
                    idxs_ap=eb_t[:, t * 64:(t + 1) * 64],
                    num_idxs=EPT, num_idxs_reg=EPT, elem_size=D)
                # stacked transpose: [128, 128] f32 blocks pair chunks
                # (2u, 2u+1) into top/bottom feature halves.
                tp = tpps.tile([128, 512], f32)
                for u in range(4):
                    nc.tensor.transpose(
                        out=tp[:, 128 * u:128 * (u + 1)],
                        in_=ge[:, 2 * u * D:(2 * u + 2) * D],
                        identity=id_t[:])
                eut_sb = eut.tile([128, 512], bf16)
                nc.scalar.copy(eut_sb[:], tp[:])
                euts.append(eut_sb)

                h1p = mmps.tile([128, 512], f32, tag="mm")
                nc.tensor.matmul(h1p[:], lhsT=(w1a_t[:]),
                                 rhs=(eut_sb[:]), start=True, stop=False)
                nc.tensor.matmul(h1p[:], lhsT=(qT2_sb[:]),
                                 rhs=(ind_t[:, t * 512:(t + 1) * 512]),
                                 start=False, stop=True)
                h1sb = hsb.tile([128, 512], bf16, tag="h")
                nc.scalar.activation(h1sb[:], h1p[:], AF.Relu)
                h2p = mmps.tile([128, 512], f32, tag="mm")
                nc.tensor.matmul(h2p[:], lhsT=(w2_t[:]),
                                 rhs=(h1sb[:]), start=True, stop=True)
                h2sb = hsb.tile([128, 512], bf16, tag="h")
                nc.scalar.activation(h2sb[:], h2p[:], AF.Relu, bias=b2_t[:, :1])
                nc.tensor.matmul(lg8[:], lhsT=(w3_t[:, 8 * t:8 * (t + 1)]),
                                 rhs=(h2sb[:]), start=(t == 0),
                                 stop=(t == TPB - 1))

            # ---- softmax over each node's 32 edges, in [8, (j k)] layout ----
            mx8 = nm.tile([8, 16, 1], f32, tag="mx8")
            nc.vector.tensor_reduce(
                out=mx8[:, :, 0], in_=lg8[:].rearrange("p (j k) -> p j k", j=16),
                axis=AX.X, op=ALU.max, negate=True)
            sub8 = nm.tile([8, 512], f32, tag="sub8")
            in0b, in1b = bass.broadcast_tensor_aps(
                lg8[:].rearrange("p (j k) -> p j k", j=16), mx8[:])
            nc.vector.tensor_tensor(
                out=sub8[:].rearrange("p (j k) -> p j k", j=16),
                in0=in0b, in1=in1b, op=ALU.add)
            e8 = nm.tile([8, 512], f32, tag="e8")
            nc.scalar.activation(e8[:], sub8[:], AF.Exp)
            s8 = nm.tile([8, 16, 1], f32, tag="s8")
            nc.vector.tensor_reduce(
                out=s8[:, :, 0], in_=e8[:].rearrange("p (j k) -> p j k", j=16),
                axis=AX.X, op=ALU.add)
            r8 = nm.tile([8, 16, 1], f32, tag="r8")
            nc.vector.reciprocal(r8[:], s8[:])
            att8 = nm.tile([8, 512], bf16, tag="att8")
            ain0, ain1 = bass.broadcast_tensor_aps(
                e8[:].rearrange("p (j k) -> p j k", j=16), r8[:])
            nc.vector.tensor_tensor(
                out=att8[:].rearrange("p (j k) -> p j k", j=16),
                in0=ain0, in1=ain1, op=ALU.mult)

            # ---- weighted aggregation: ones8[q, t*128+p] = (q == 2t+p//64)
            # broadcasts att8 row 2t+h to partition half h of tile t.
            wacc = wacc_p.tile([128, D], f32)
            for t in range(TPB):
                ab = abps.tile([128, 512], f32)
                nc.tensor.matmul(ab[:], lhsT=(ones_t[:, t * 128:(t + 1) * 128]),
                                 rhs=(att8[:]), start=True, stop=True)
                wt = wsb_p.tile([128, 512], f32)
                nc.vector.tensor_tensor(out=wt[:], in0=euts[t][:], in1=ab[:],
                                        op=ALU.mult)
                nc.vector.tensor_reduce(
                    out=wacc[:, 16 * t:16 * (t + 1)],
                    in_=wt[:].rearrange("p (j k) -> p j k", j=16),
                    axis=AX.X, op=ALU.add)
            outp = qps.tile([128, 128], f32, tag="qp")
            nc.tensor.transpose(out=outp[0:64, :], in_=wacc[:], identity=id_t[:])
            osb = osb_p.tile([64, 128], f32)
            nc.scalar.copy(osb[:], outp[0:64, :])
            nc.sync.dma_start(
                outd[b * 128:(b + 1) * 128, :]
                    .rearrange("(pair n) d -> n pair d", pair=2),
                osb[:].rearrange("n (pair d) -> n pair d", pair=2))

    nc.compile()
    return nc


def _wrap16(arr):
    """[n] -> [128, n/16] int16 idx layout (16-partition wrap, replicated)."""
    n = arr.shape[0]
    w = arr.reshape(n // 16, 16).T.astype(np.int16)
    return np.ascontiguousarray(np.tile(w, (8, 1)))


def _prep_host(nodes, neigh_idx, att1_w, att1_b, att2_w, att2_b, att3_w,
               nblk_per_core):
    """Shard + reorder indices, build constant tensors. Returns per-core maps
    (without the shared tables) and the stage-A pad size pa."""
    npad = nblk_per_core * 128
    npc = min(NPC, npad)
    nodes = np.asarray(nodes).astype(np.int32)
    neigh = np.asarray(neigh_idx).astype(np.int32).reshape(-1, DEG)
    groups = _groups(nblk_per_core)
    ng = len(groups)

    consts = {}
    att1_w = np.asarray(att1_w, np.float32)
    w1aT = att1_w[:, :D].T.copy()
    w1bT = att1_w[:, D:].T.copy()
    w2T = np.asarray(att2_w, np.float32).T.copy()

    def blockdiag(m):
        z = np.zeros((128, 128), np.float32)
        z[:64, :64] = m
        z[64:, 64:] = m
        return z

    import ml_dtypes
    bf = ml_dtypes.bfloat16
    consts["w1a"] = blockdiag(w1aT).astype(bf)
    consts["w1b"] = blockdiag(w1bT)
    consts["w2"] = blockdiag(w2T).astype(bf)
    # w3q[:, t*8 + 2t + h] = w3 half-h; tile t's mm3 writes lg8 rows 2t, 2t+1
    w3q = np.zeros((128, TPB, 8), np.float32)
    w3row = np.asarray(att3_w, np.float32)[0]
    for t in range(TPB):
        w3q[:64, t, 2 * t] = w3row
        w3q[64:, t, 2 * t + 1] = w3row
    consts["w3q"] = w3q.reshape(128, TPB * 8).astype(bf)
    ones8 = np.zeros((8, TPB, 128), np.float32)
    for t in range(TPB):
        ones8[2 * t, t, :64] = 1.0
        ones8[2 * t + 1, t, 64:] = 1.0
    consts["ones8"] = ones8.reshape(8, TPB * 128).astype(bf)
    # ind64[j, t*512 + e] = 1 iff j == 16t + e//32 (mm1b scatters per-node q)
    ind64 = np.zeros((64, TPB * 512), np.float32)
    for t in range(TPB):
        ind64[16 * t:16 * (t + 1), 512 * t:512 * (t + 1)] = np.repeat(
            np.eye(16, dtype=np.float32), 32, axis=1)
    consts["ind64"] = ind64.astype(bf)
    consts["b1st"] = np.tile(np.asarray(att1_b, np.float32), 2)[:, None].copy()
    consts["b2st"] = np.tile(np.asarray(att2_b, np.float32), 2)[:, None].copy()
    consts["ident"] = np.eye(128, dtype=np.float32)

    ncores = len(nodes) // npc if len(nodes) >= npc else 1

    # ---- per-core slot->uid map in gather-position order ----
    # slot order within a block: position = (8t + 2u4 + h)*128 + p, matching
    # the in-tile layout x = c*128 + p with c = 2*u4 + h.
    def slot_uids(nix):
        a = nix.reshape(nblk_per_core, 2, TPB, 16, DEG).transpose(0, 2, 1, 3, 4)
        a = a.reshape(nblk_per_core, TPB, 2, 4, 128)  # [b, t, h, u4, p]
        a = a.transpose(0, 1, 3, 2, 4)                # [b, t, u4, h, p]
        return np.ascontiguousarray(a).reshape(-1)

    # first pass: compute pa (max per-(core,group,chunk) count, padded)
    pa = 0
    per_core_uids = []
    for c in range(ncores):
        n0 = c * npc
        nix = np.zeros((npad, DEG), np.int32)
        nix[:npc] = neigh[n0:n0 + npc]
        uids = slot_uids(nix)
        per_core_uids.append(uids)
        for g, blocks in enumerate(groups):
            ge = uids[blocks[0] * TPB * EPT:
                      (blocks[-1] + 1) * TPB * EPT]
            cnt = np.bincount(ge // UCH, minlength=4).max()
            pa = max(pa, int(cnt))
    pa = ((pa + 1023) // 1024) * 1024  # dma_gather cap: 1024 idx per call
    sc = pa // 128

    per_core = []
    for c in range(ncores):
        n0 = c * npc
        gid = np.zeros(npad, np.int32)
        gid[:npc] = nodes[n0:n0 + npc]
        uids = per_core_uids[c]

        ea = np.zeros((ng * 128, 4 * (pa // 16)), np.int16)
        eb = np.zeros((nblk_per_core * 128, TPB * EPT // 16), np.int16)
        for g, blocks in enumerate(groups):
            e0 = blocks[0] * TPB * EPT
            e1 = (blocks[-1] + 1) * TPB * EPT
            u = uids[e0:e1]
            ch = u // UCH
            rel = (u % UCH).astype(np.int16)
            order = np.argsort(ch, kind="stable")
            ranks = np.empty(u.shape[0], np.int64)
            ranks[order] = np.arange(u.shape[0])
            counts = np.bincount(ch, minlength=4)
            cum = np.concatenate([[0], np.cumsum(counts)])
            r_in = ranks - cum[ch]
            gpos = ch * pa + r_in            # padded sorted position
            ea_list = np.zeros(4 * pa, np.int16)
            ea_list[gpos] = rel
            # scratch row of padded sorted position s:
            j = gpos % pa
            cpos = gpos // pa
            srow = (j % 128) * (4 * sc) + cpos * sc + j // 128
            ea[g * 128:(g + 1) * 128] = _wrap16(ea_list)
            srow = srow.reshape(len(blocks), TPB * EPT)
            for bi, b in enumerate(blocks):
                eb[b * 128:(b + 1) * 128] = _wrap16(srow[bi].astype(np.int16))

        # q pair idx: per group, position 128*(2*bg+h) + p -> pair id of
        # node 64*h + p%64 of block b (p >= 64 duplicates p-64).
        gq = np.zeros((ng * 128, 16 * NBG), np.int16)
        qmask = np.zeros((64, nblk_per_core, 4), np.float32)
        gid_b = gid.reshape(nblk_per_core, 2, 64)  # [b, h, j]
        for g, blocks in enumerate(groups):
            gb = len(blocks)
            ql = np.zeros((2 * gb) * 128, np.int16)
            for bi, b in enumerate(blocks):
                for h in range(2):
                    col = gid_b[b, h] // 2
                    ql[(2 * bi + h) * 128:(2 * bi + h) * 128 + 64] = col
                    ql[(2 * bi + h) * 128 + 64:(2 * bi + h + 1) * 128] = col
            gq[g * 128:(g + 1) * 128, 0:16 * gb] = _wrap16(ql)
        for b in range(nblk_per_core):
            for h in range(2):
                par = (gid_b[b, h] % 2).astype(np.float32)
                qmask[:, b, 2 * h] = 1.0 - par
                qmask[:, b, 2 * h + 1] = par

        m = dict(consts)
        m["ea"] = np.ascontiguousarray(ea)
        m["eb"] = np.ascontiguousarray(eb)
        m["gq"] = np.ascontiguousarray(gq)
        m["qm"] = np.ascontiguousarray(qmask.reshape(64, nblk_per_core * 4))
        per_core.append(m)
    return per_core, pa


def kernel(nodes, neigh_idx, segment_ids, u2e_weight, g2e_weight,
           att1_w, att1_b, att2_w, att2_b, att3_w, att3_b):
    from concourse import bass_utils

    nblk = NPC // 128 + (1 if NPC % 128 else 0)  # 40
    per_core, pa = _prep_host(nodes, neigh_idx, att1_w, att1_b, att2_w,
                              att2_b, att3_w, nblk)
    key = ("prog", nblk, pa)
    if key not in _cache:
        _cache[key] = _build_program(nblk, pa)
    nc = _cache[key]

    u2ef = np.ascontiguousarray(np.asarray(u2e_weight, np.float32))
    g2ep = np.ascontiguousarray(
        np.asarray(g2e_weight, np.float32).reshape(NUM_GROUPS // 2, 2 * D))
    in_maps = []
    for m in per_core:
        m = dict(m)
        m["u2ef"] = u2ef
        m["g2ep"] = g2ep
        in_maps.append(m)

    res = bass_utils.run_bass_kernel_spmd(nc, in_maps,
                                          core_ids=list(range(N_CORES)))
    outs = [np.asarray(r["out"])[:NPC] for r in res.results]
    return np.concatenate(outs, axis=0)
